# revision 1
# baseline (speedup 1.0000x reference)
"""Trainium2 Bass kernel for AttentionConvFull (local 5x5 window attention
with per-channel softmax, grouped 1x1 conv projections).

Sharding: 8 cores = batch(4) x H-halves(2). Each core gets a 32-row halo'd,
zero-padded slice of x, pre-transposed on host to channel-major [256, 32*60].
No collectives needed.

Per-core dataflow (2 channel-chunks of 128 partitions each):
  PE    : block-diag 128x128 bf16 matmuls for q/k/v projections; per window
          offset j, identity-matmul PSUM accumulation of den += e_j and
          num += (e_j * v_j)  (bf16 inputs, fp32 accumulate). Redundant
          identity weight reloads stripped post-compile.
  DMA   : 1-element-shifted copies of the k/v maps (contiguous SBUF->SBUF)
          so odd window columns keep 4B alignment for the DVE 2x/4x modes
  DVE   : t = kr * q (flat), w = e * v_view (2D window view);
          ~40% of the kr = k_view + rel_j adds (tensor_scalar)
  ACT   : e = exp(t); ~60% of the kr adds (Identity + per-partition bias);
          projection PSUM->SBUF casts (+q_emb bias fused for q)
  Epilogue: out = num * recip_approx(den), DMA out channel-major; host
  reassembles to (B,H,W,C).
"""

import numpy as np
import ml_dtypes

import concourse.bass as bass
import concourse.tile as tile
from concourse import bacc, mybir
from concourse.bass_utils import run_bass_kernel_spmd

F32 = mybir.dt.float32
BF16 = mybir.dt.bfloat16

K = 5
G = 8
B, H, W, C = 4, 56, 56, 256
Cg = C // G            # 32
P = K // 2             # 2
HS = H // 2            # 28 output rows per shard
MR = HS + 2 * P        # 32 map rows
MC = W + 2 * P         # 60 map cols
SP = MR * MC           # 1920 map spatial
OP = HS * W            # 1568 output spatial per shard
NCH = 2                # channel chunks of 128 partitions
NCORES = 8
HALF = OP // 2         # 784: PSUM accumulate tile half-size


def _dedup_ldweights(nc):
    """Remove redundant PE weight reloads: consecutive InstLdweights that
    load the same stationary operand with no sync info. The identity matrix
    stays resident across the whole accumulation loop, so only the first
    load is needed. Self-loading matmuls (bf16 projections) reset the
    tracked weight state."""
    removed = 0
    for blk in nc.main_func.blocks:
        last_sig = None
        keep = []
        for inst in blk.instructions:
            if isinstance(inst, mybir.InstLdweights):
                sig = " ".join(a.concise() for a in inst.ins)
                si = inst.sync_info
                clean = si is None or (
                    len(si.on_wait) == 0 and len(si.on_update) == 0
                )
                if sig == last_sig and clean:
                    removed += 1
                    continue
                last_sig = sig
            elif isinstance(inst, mybir.InstMatmult):
                if len(inst.ins) > 1:
                    wsig = inst.ins[1].concise()
                    if wsig != last_sig:
                        last_sig = wsig
            keep.append(inst)
        blk.instructions[:] = keep
    return removed


def build_nc():
    nc = bacc.Bacc(
        "TRN2", target_bir_lowering=False, debug=False, num_devices=NCORES
    )

    xt_d = nc.dram_tensor("xt", [NCH, 128, SP], BF16, kind="ExternalInput").ap()
    wq_d = nc.dram_tensor("wqb", [NCH, 128, 128], BF16, kind="ExternalInput").ap()
    wk_d = nc.dram_tensor("wkb", [NCH, 128, 128], BF16, kind="ExternalInput").ap()
    wv_d = nc.dram_tensor("wvb", [NCH, 128, 128], BF16, kind="ExternalInput").ap()
    rel_d = nc.dram_tensor("relb", [NCH, 128, K * K], F32, kind="ExternalInput").ap()
    qe_d = nc.dram_tensor("qeb", [NCH, 128, 1], F32, kind="ExternalInput").ap()
    id_d = nc.dram_tensor("idn", [128, 128], BF16, kind="ExternalInput").ap()
    out_d = nc.dram_tensor("out", [NCH, 128, OP], F32, kind="ExternalOutput").ap()

    with tile.TileContext(nc) as tc:
        with (
            tc.tile_pool(name="consts", bufs=1) as consts,
            tc.tile_pool(name="weights", bufs=2) as wpool,
            tc.tile_pool(name="xin", bufs=2) as xpool,
            tc.tile_pool(name="maps", bufs=2) as mpool,
            tc.tile_pool(name="jwork", bufs=4) as jpool,
            tc.tile_pool(name="epi", bufs=2) as epool,
            tc.tile_pool(name="acc", bufs=4, space=bass.MemorySpace.PSUM) as psum,
        ):
            ident = consts.tile([128, 128], BF16, tag="ident")
            nc.sync.dma_start(ident[:], id_d)

            kmaps, komaps, vmaps, vomaps, qflats, rels = [], [], [], [], [], []

            for c in range(NCH):
                x_sb = xpool.tile([128, SP], BF16, tag="x")
                nc.sync.dma_start(x_sb[:], xt_d[c])

                wts = {}
                for nm, d in (("wq", wq_d), ("wk", wk_d), ("wv", wv_d)):
                    t = wpool.tile([128, 128], BF16, tag=nm, name=f"{nm}{c}")
                    nc.sync.dma_start(t[:], d[c])
                    wts[nm] = t
                rel_sb = wpool.tile([128, K * K], F32, tag="rel", name=f"rel{c}")
                nc.sync.dma_start(rel_sb[:], rel_d[c])
                qe_sb = wpool.tile([128, 1], F32, tag="qe", name=f"qe{c}")
                nc.sync.dma_start(qe_sb[:], qe_d[c])
                rels.append(rel_sb)

                k_bf = mpool.tile([128, SP], BF16, tag="k", name=f"k{c}")
                v_bf = mpool.tile([128, SP], BF16, tag="v", name=f"v{c}")
                qf = mpool.tile([128, OP], BF16, tag="qf", name=f"qf{c}")

                # projections: 2 psum tiles of 960 cols (16 map rows) each
                NS = 2
                SL = SP // NS  # 960
                for s in range(NS):
                    lo = s * SL
                    rhs = x_sb[:, lo : lo + SL]
                    for nm in ("wk", "wv", "wq"):
                        ps = psum.tile(
                            [128, SL], F32, tag="acc", name=f"pp{c}{s}{nm}"
                        )
                        for mlo, mn in ((0, 512), (512, SL - 512)):
                            nc.tensor.matmul(
                                ps[:, mlo : mlo + mn],
                                wts[nm][:],
                                rhs[:, mlo : mlo + mn],
                                start=True,
                                stop=True,
                            )
                        if nm == "wq":
                            # write the interior (h in [2,30), w in [2,58))
                            # of this 16-row band directly into flat q,
                            # fusing the q_emb per-partition bias
                            r0 = max(P, 16 * s)
                            r1 = min(MR - P, 16 * (s + 1))
                            src = ps[:].rearrange("p (h w) -> p h w", h=16)[
                                :, r0 - 16 * s : r1 - 16 * s, P : P + W
                            ]
                            dst = qf[:].rearrange("p (h w) -> p h w", h=HS)[
                                :, r0 - P : r1 - P, :
                            ]
                            nc.scalar.activation(
                                dst,
                                src,
                                mybir.ActivationFunctionType.Identity,
                                bias=qe_sb[:],
                            )
                        elif nm == "wk":
                            nc.scalar.copy(k_bf[:, lo : lo + SL], ps[:])
                        else:
                            nc.scalar.copy(v_bf[:, lo : lo + SL], ps[:])

                # 1-elem-shifted copies (contiguous SBUF->SBUF DMA, cheap):
                # x_od[i] = x[i+1], so odd-dj window reads stay 4B-aligned
                k_od = mpool.tile([128, SP], BF16, tag="ko", name=f"ko{c}")
                v_od = mpool.tile([128, SP], BF16, tag="vo", name=f"vo{c}")
                nc.sync.dma_start(k_od[:, : SP - 1], k_bf[:, 1:])
                nc.sync.dma_start(v_od[:, : SP - 1], v_bf[:, 1:])

                kmaps.append(k_bf); komaps.append(k_od)
                vmaps.append(v_bf); vomaps.append(v_od)
                qflats.append(qf)

            # ---- j-loop per chunk ----
            for c in range(NCH):
                rel_sb, qf = rels[c], qflats[c]

                den = [
                    psum.tile([128, HALF], F32, tag="acc", name=f"den{c}{h}")
                    for h in range(2)
                ]
                num = [
                    psum.tile([128, HALF], F32, tag="acc", name=f"num{c}{h}")
                    for h in range(2)
                ]

                # dj-major order: the odd-shifted maps (needed from dj=1)
                # arrive via DMA while the dj=0 iterations run
                for dj in range(K):
                  for di in range(K):
                    j = di * K + dj
                    if dj % 2 == 0:
                        ksrc, vsrc, dje = kmaps[c], vmaps[c], dj
                    else:
                        ksrc, vsrc, dje = komaps[c], vomaps[c], dj - 1
                    k3 = ksrc[:].rearrange("p (h w) -> p h w", h=MR)
                    v3 = vsrc[:].rearrange("p (h w) -> p h w", h=MR)
                    kv = k3[:, di : di + HS, dje : dje + W]
                    vv = v3[:, di : di + HS, dje : dje + W]

                    kr_t = jpool.tile([128, OP], BF16, tag="kr", name=f"kr{c}{j}")
                    kr3 = kr_t[:].rearrange("p (h w) -> p h w", h=HS)
                    # balance the rel-add between ACT and DVE
                    if (3 * j + c) % 5 < 3:
                        nc.scalar.activation(
                            kr3,
                            kv,
                            mybir.ActivationFunctionType.Identity,
                            bias=rel_sb[:, j : j + 1],
                        )
                    else:
                        nc.vector.tensor_scalar(
                            kr3,
                            kv,
                            rel_sb[:, j : j + 1],
                            None,
                            mybir.AluOpType.add,
                        )

                    t_t = jpool.tile([128, OP], BF16, tag="t", name=f"t{c}{j}")
                    nc.vector.tensor_tensor(
                        t_t[:], kr_t[:], qf[:], mybir.AluOpType.mult
                    )

                    e_t = jpool.tile([128, OP], BF16, tag="e", name=f"e{c}{j}")
                    nc.scalar.activation(
                        e_t[:], t_t[:], mybir.ActivationFunctionType.Exp
                    )

                    w_t = jpool.tile([128, OP], BF16, tag="w", name=f"w{c}{j}")
                    w3 = w_t[:].rearrange("p (h w) -> p h w", h=HS)
                    e3 = e_t[:].rearrange("p (h w) -> p h w", h=HS)
                    nc.vector.tensor_tensor(w3, e3, vv, mybir.AluOpType.mult)

                    st = j == 0
                    sp = j == K * K - 1
                    # all den matmuls (one wait on e_t), then all num (w_t):
                    # keeps PE back-to-back instead of isolated-drain per MM
                    for acc, src_t in ((den, e_t), (num, w_t)):
                        for h in range(2):
                            base = h * HALF
                            for lo, n in ((0, 512), (512, HALF - 512)):
                                nc.tensor.matmul(
                                    acc[h][:, lo : lo + n],
                                    ident[:],
                                    src_t[:, base + lo : base + lo + n],
                                    start=st,
                                    stop=sp,
                                )

                # ---- epilogue ----
                out_sb = epool.tile([128, OP], F32, tag="osb", name=f"osb{c}")
                for h in range(2):
                    base = h * HALF
                    rden = epool.tile([128, HALF], F32, tag="rden", name=f"rd{c}{h}")
                    nc.vector.reciprocal_approx_fast(rden[:], den[h][:])
                    nc.vector.tensor_tensor(
                        out_sb[:, base : base + HALF],
                        num[h][:],
                        rden[:],
                        mybir.AluOpType.mult,
                    )
                nc.sync.dma_start(out_d[c], out_sb[:])

    nc.compile()
    _dedup_ldweights(nc)
    return nc


def _block_diag_weights(w):
    """w: (G, Cg_out, Cg_in) -> lhsT layout [NCH, 128, 128] where
    lhsT[c, ci, co] = w[g, co%32, ci%32] for matching 32-blocks."""
    out = np.zeros((NCH, 128, 128), np.float32)
    for c in range(NCH):
        for g4 in range(4):
            g = c * 4 + g4
            blk = w[g]  # (Cg_out, Cg_in)
            out[c, g4 * 32 : (g4 + 1) * 32, g4 * 32 : (g4 + 1) * 32] = blk.T
    return out


_NC_CACHE = {}


def _make_in_maps(inputs):
    x = np.asarray(inputs["x"], np.float32)
    wq = np.asarray(inputs["wq"], np.float32)
    wk = np.asarray(inputs["wk"], np.float32)
    wv = np.asarray(inputs["wv"], np.float32)
    rel_emb = np.asarray(inputs["rel_emb"], np.float32)
    q_emb = np.asarray(inputs["q_emb"], np.float32)

    bf = ml_dtypes.bfloat16
    wqb = _block_diag_weights(wq).astype(bf)
    wkb = _block_diag_weights(wk).astype(bf)
    wvb = _block_diag_weights(wv).astype(bf)
    relb = np.ascontiguousarray(
        rel_emb.reshape(G, Cg, K * K).reshape(NCH, 128, K * K)
    )
    qeb = np.ascontiguousarray(q_emb.reshape(NCH, 128, 1))
    idn = np.eye(128, dtype=bf)

    xp = np.pad(x, ((0, 0), (P, P), (P, P), (0, 0)))  # (B, 60, 60, C)

    in_maps = []
    for core in range(NCORES):
        b, half = divmod(core, 2)
        sh = xp[b, HS * half : HS * half + MR]         # (32, 60, C)
        xt = np.ascontiguousarray(sh.reshape(SP, C).T).reshape(NCH, 128, SP)
        in_maps.append(
            {
                "xt": xt.astype(bf),
                "wqb": wqb,
                "wkb": wkb,
                "wvb": wvb,
                "relb": relb,
                "qeb": qeb,
                "idn": idn,
            }
        )
    return in_maps


def kernel(**inputs):
    in_maps = _make_in_maps(inputs)

    if "nc" not in _NC_CACHE:
        _NC_CACHE["nc"] = build_nc()
    nc = _NC_CACHE["nc"]

    res = run_bass_kernel_spmd(nc, in_maps, core_ids=list(range(NCORES)))

    out = np.empty((B, H, W, C), np.float32)
    for core in range(NCORES):
        b, half = divmod(core, 2)
        o = res.results[core]["out"].reshape(C, HS, W)
        out[b, HS * half : HS * half + HS] = o.transpose(1, 2, 0)
    return out



# revision 4
# speedup vs baseline: 1.0146x; 1.0146x over previous
"""Trainium2 Bass kernel for AttentionConvFull (local 5x5 window attention
with per-channel softmax, grouped 1x1 conv projections).

Sharding: 8 cores = batch(4) x H-halves(2). Each core gets a 32-row halo'd,
zero-padded slice of x, pre-transposed on host to channel-major [256, 32*60].
No collectives needed.

V2 dataflow per core (2 channel-chunks of 128 partitions each):
  PE    : block-diag 128x128 bf16 matmuls for q/k/v projections; per window
          offset j, identity-matmul PSUM accumulation of den += e_j and
          num += (e_j * v_j).
  DVE   : fused scalar_tensor_tensor t_j = (k_j + rel_j) * q (one pass,
          replaces the separate rel-add); w_j = e_j * v_j tensor_tensor;
          PSUM->SBUF projection copies.
  ACT   : exp over j-PAIRS (batched to amortize per-instr overhead); q
          projection copy with fused q_emb bias.
  GPSIMD: takes a subset of the w_j multiplies to relieve DVE.
  DMA   : 1-elem-shifted copies of k/v maps so odd window columns keep 4B
          alignment (DVE 2x mode); bf16 output (host upcasts to f32).
"""

import numpy as np
import ml_dtypes

import concourse.bass as bass
import concourse.tile as tile
from concourse import bacc, mybir
from concourse.bass_utils import run_bass_kernel_spmd

F32 = mybir.dt.float32
BF16 = mybir.dt.bfloat16

K = 5
G = 8
B, H, W, C = 4, 56, 56, 256
Cg = C // G            # 32
P = K // 2             # 2
HS = H // 2            # 28 output rows per shard
MR = HS + 2 * P        # 32 map rows
MC = W + 2 * P         # 60 map cols
SP = MR * MC           # 1920 map spatial
OP = HS * W            # 1568 output spatial per shard
NCH = 2                # channel chunks of 128 partitions
NCORES = 8
HALF = OP // 2         # 784: PSUM accumulate tile half-size

# which j-iterations (0..24, dj-major order position) run their w-multiply
# on GPSIMD instead of DVE
GP_POS = {2, 7, 12, 17, 22}


def _dedup_ldweights(nc):
    """Remove redundant PE weight reloads: consecutive InstLdweights that
    load the same stationary operand with no sync info."""
    removed = 0
    for blk in nc.main_func.blocks:
        last_sig = None
        keep = []
        for inst in blk.instructions:
            if isinstance(inst, mybir.InstLdweights):
                sig = " ".join(a.concise() for a in inst.ins)
                si = inst.sync_info
                clean = si is None or (
                    len(si.on_wait) == 0 and len(si.on_update) == 0
                )
                if sig == last_sig and clean:
                    removed += 1
                    continue
                last_sig = sig
            elif isinstance(inst, mybir.InstMatmult):
                if len(inst.ins) > 1:
                    wsig = inst.ins[1].concise()
                    if wsig != last_sig:
                        last_sig = wsig
            keep.append(inst)
        blk.instructions[:] = keep
    return removed


def build_nc():
    nc = bacc.Bacc(
        "TRN2", target_bir_lowering=False, debug=False, num_devices=NCORES
    )

    xt_d = nc.dram_tensor("xt", [NCH, 128, SP], BF16, kind="ExternalInput").ap()
    wq_d = nc.dram_tensor("wqb", [NCH, 128, 128], BF16, kind="ExternalInput").ap()
    wk_d = nc.dram_tensor("wkb", [NCH, 128, 128], BF16, kind="ExternalInput").ap()
    wv_d = nc.dram_tensor("wvb", [NCH, 128, 128], BF16, kind="ExternalInput").ap()
    rel_d = nc.dram_tensor("relb", [NCH, 128, K * K], F32, kind="ExternalInput").ap()
    qe_d = nc.dram_tensor("qeb", [NCH, 128, 1], F32, kind="ExternalInput").ap()
    id_d = nc.dram_tensor("idn", [128, 128], BF16, kind="ExternalInput").ap()
    out_d = nc.dram_tensor("out", [NCH, 128, OP], BF16, kind="ExternalOutput").ap()

    # dj-major j order: odd-shifted maps (needed from dj=1) arrive via DMA
    # while dj=0 iterations run
    JLIST = [(di, dj) for dj in range(K) for di in range(K)]

    with tile.TileContext(nc) as tc:
        with (
            tc.tile_pool(name="consts", bufs=1) as consts,
            tc.tile_pool(name="weights", bufs=2) as wpool,
            tc.tile_pool(name="xin", bufs=2) as xpool,
            tc.tile_pool(name="maps", bufs=2) as mpool,
            tc.tile_pool(name="jwork", bufs=3) as jpool,
            tc.tile_pool(name="epi", bufs=2) as epool,
            tc.tile_pool(name="acc", bufs=4, space=bass.MemorySpace.PSUM) as psum,
        ):
            ident = consts.tile([128, 128], BF16, tag="ident")
            nc.sync.dma_start(ident[:], id_d)

            kmaps, komaps, vmaps, vomaps, qflats, rels = [], [], [], [], [], []

            for c in range(NCH):
                x_sb = xpool.tile([128, SP], BF16, tag="x")
                nc.sync.dma_start(x_sb[:], xt_d[c])

                wts = {}
                for nm, d in (("wq", wq_d), ("wk", wk_d), ("wv", wv_d)):
                    t = wpool.tile([128, 128], BF16, tag=nm, name=f"{nm}{c}")
                    nc.sync.dma_start(t[:], d[c])
                    wts[nm] = t
                rel_sb = wpool.tile([128, K * K], F32, tag="rel", name=f"rel{c}")
                nc.sync.dma_start(rel_sb[:], rel_d[c])
                qe_sb = wpool.tile([128, 1], F32, tag="qe", name=f"qe{c}")
                nc.sync.dma_start(qe_sb[:], qe_d[c])
                rels.append(rel_sb)

                k_bf = mpool.tile([128, SP], BF16, tag="k", name=f"k{c}")
                v_bf = mpool.tile([128, SP], BF16, tag="v", name=f"v{c}")
                qf = mpool.tile([128, OP], BF16, tag="qf", name=f"qf{c}")

                # projections: weight-major to minimize ldweights; 2 psum
                # tiles of 960 cols (16 map rows) each
                NS = 2
                SL = SP // NS  # 960
                for nm in ("wk", "wv", "wq"):
                    for s in range(NS):
                        lo = s * SL
                        rhs = x_sb[:, lo : lo + SL]
                        ps = psum.tile(
                            [128, SL], F32, tag="acc", name=f"pp{c}{s}{nm}"
                        )
                        for mlo, mn in ((0, 512), (512, SL - 512)):
                            nc.tensor.matmul(
                                ps[:, mlo : mlo + mn],
                                wts[nm][:],
                                rhs[:, mlo : mlo + mn],
                                start=True,
                                stop=True,
                            )
                        if nm == "wq":
                            # write the interior (h in [2,30), w in [2,58))
                            # of this 16-row band directly into flat q,
                            # fusing the q_emb per-partition bias (ACT)
                            r0 = max(P, 16 * s)
                            r1 = min(MR - P, 16 * (s + 1))
                            src = ps[:].rearrange("p (h w) -> p h w", h=16)[
                                :, r0 - 16 * s : r1 - 16 * s, P : P + W
                            ]
                            dst = qf[:].rearrange("p (h w) -> p h w", h=HS)[
                                :, r0 - P : r1 - P, :
                            ]
                            nc.scalar.activation(
                                dst,
                                src,
                                mybir.ActivationFunctionType.Identity,
                                bias=qe_sb[:],
                            )
                        elif nm == "wk":
                            nc.vector.tensor_copy(k_bf[:, lo : lo + SL], ps[:])
                        else:
                            nc.vector.tensor_copy(v_bf[:, lo : lo + SL], ps[:])

                # 1-elem-shifted copies (contiguous SBUF->SBUF DMA):
                # x_od[i] = x[i+1], so odd-dj window reads stay 4B-aligned
                k_od = mpool.tile([128, SP], BF16, tag="ko", name=f"ko{c}")
                v_od = mpool.tile([128, SP], BF16, tag="vo", name=f"vo{c}")
                nc.sync.dma_start(k_od[:, : SP - 1], k_bf[:, 1:])
                nc.sync.dma_start(v_od[:, : SP - 1], v_bf[:, 1:])

                kmaps.append(k_bf); komaps.append(k_od)
                vmaps.append(v_bf); vomaps.append(v_od)
                qflats.append(qf)

            # ---- j-loop per chunk ----
            for c in range(NCH):
                rel_sb, qf = rels[c], qflats[c]
                qf3 = qf[:].rearrange("p (h w) -> p h w", h=HS)

                den = [
                    psum.tile([128, HALF], F32, tag="acc", name=f"den{c}{h}")
                    for h in range(2)
                ]
                num = [
                    psum.tile([128, HALF], F32, tag="acc", name=f"num{c}{h}")
                    for h in range(2)
                ]

                # pairs of consecutive j positions share one exp instruction
                pairs = [JLIST[i : i + 2] for i in range(0, len(JLIST), 2)]
                pos = 0
                for pr in pairs:
                    npr = len(pr)
                    t2 = jpool.tile(
                        [128, npr * OP], BF16, tag="t2", name=f"t{c}{pos}"
                    )
                    e2 = jpool.tile(
                        [128, npr * OP], BF16, tag="e2", name=f"e{c}{pos}"
                    )
                    vviews = []
                    # stt: t = (k_j + rel_j) * q, one DVE pass per j
                    for i, (di, dj) in enumerate(pr):
                        j = di * K + dj
                        if dj % 2 == 0:
                            ksrc, vsrc, dje = kmaps[c], vmaps[c], dj
                        else:
                            ksrc, vsrc, dje = komaps[c], vomaps[c], dj - 1
                        k3 = ksrc[:].rearrange("p (h w) -> p h w", h=MR)
                        v3 = vsrc[:].rearrange("p (h w) -> p h w", h=MR)
                        kv = k3[:, di : di + HS, dje : dje + W]
                        vviews.append(v3[:, di : di + HS, dje : dje + W])
                        t3 = t2[:, i * OP : (i + 1) * OP].rearrange(
                            "p (h w) -> p h w", h=HS
                        )
                        nc.vector.scalar_tensor_tensor(
                            t3,
                            kv,
                            rel_sb[:, j : j + 1],
                            qf3,
                            mybir.AluOpType.add,
                            mybir.AluOpType.mult,
                        )

                    # one exp pass for the pair
                    nc.scalar.activation(
                        e2[:], t2[:], mybir.ActivationFunctionType.Exp
                    )

                    for i, (di, dj) in enumerate(pr):
                        eflat = e2[:, i * OP : (i + 1) * OP]
                        e3 = eflat.rearrange("p (h w) -> p h w", h=HS)
                        w_t = jpool.tile(
                            [128, OP], BF16, tag="w", name=f"w{c}{pos + i}"
                        )
                        w3 = w_t[:].rearrange("p (h w) -> p h w", h=HS)
                        eng = (
                            nc.gpsimd if (pos + i) in GP_POS else nc.vector
                        )
                        eng.tensor_tensor(
                            w3, e3, vviews[i], mybir.AluOpType.mult
                        )

                        st = pos + i == 0
                        sp = pos + i == K * K - 1
                        for acc, src_t in ((den, eflat), (num, w_t[:])):
                            for h in range(2):
                                base = h * HALF
                                for lo, n in ((0, 512), (512, HALF - 512)):
                                    nc.tensor.matmul(
                                        acc[h][:, lo : lo + n],
                                        ident[:],
                                        src_t[:, base + lo : base + lo + n],
                                        start=st,
                                        stop=sp,
                                    )
                    pos += npr

                # ---- epilogue ----
                out_sb = epool.tile([128, OP], BF16, tag="osb", name=f"osb{c}")
                for h in range(2):
                    base = h * HALF
                    rden = epool.tile([128, HALF], F32, tag="rden", name=f"rd{c}{h}")
                    nc.vector.reciprocal_approx_fast(rden[:], den[h][:])
                    nc.vector.tensor_tensor(
                        out_sb[:, base : base + HALF],
                        num[h][:],
                        rden[:],
                        mybir.AluOpType.mult,
                    )
                nc.sync.dma_start(out_d[c], out_sb[:])

    nc.compile()
    _dedup_ldweights(nc)
    return nc


def _block_diag_weights(w):
    """w: (G, Cg_out, Cg_in) -> lhsT layout [NCH, 128, 128] where
    lhsT[c, ci, co] = w[g, co%32, ci%32] for matching 32-blocks."""
    out = np.zeros((NCH, 128, 128), np.float32)
    for c in range(NCH):
        for g4 in range(4):
            g = c * 4 + g4
            blk = w[g]  # (Cg_out, Cg_in)
            out[c, g4 * 32 : (g4 + 1) * 32, g4 * 32 : (g4 + 1) * 32] = blk.T
    return out


_NC_CACHE = {}


def _make_in_maps(inputs):
    x = np.asarray(inputs["x"], np.float32)
    wq = np.asarray(inputs["wq"], np.float32)
    wk = np.asarray(inputs["wk"], np.float32)
    wv = np.asarray(inputs["wv"], np.float32)
    rel_emb = np.asarray(inputs["rel_emb"], np.float32)
    q_emb = np.asarray(inputs["q_emb"], np.float32)

    bf = ml_dtypes.bfloat16
    wqb = _block_diag_weights(wq).astype(bf)
    wkb = _block_diag_weights(wk).astype(bf)
    wvb = _block_diag_weights(wv).astype(bf)
    relb = np.ascontiguousarray(
        rel_emb.reshape(G, Cg, K * K).reshape(NCH, 128, K * K)
    )
    qeb = np.ascontiguousarray(q_emb.reshape(NCH, 128, 1))
    idn = np.eye(128, dtype=bf)

    xp = np.pad(x, ((0, 0), (P, P), (P, P), (0, 0)))  # (B, 60, 60, C)

    in_maps = []
    for core in range(NCORES):
        b, half = divmod(core, 2)
        sh = xp[b, HS * half : HS * half + MR]         # (32, 60, C)
        xt = np.ascontiguousarray(sh.reshape(SP, C).T).reshape(NCH, 128, SP)
        in_maps.append(
            {
                "xt": xt.astype(bf),
                "wqb": wqb,
                "wkb": wkb,
                "wvb": wvb,
                "relb": relb,
                "qeb": qeb,
                "idn": idn,
            }
        )
    return in_maps


def kernel(**inputs):
    in_maps = _make_in_maps(inputs)

    if "nc" not in _NC_CACHE:
        _NC_CACHE["nc"] = build_nc()
    nc = _NC_CACHE["nc"]

    res = run_bass_kernel_spmd(nc, in_maps, core_ids=list(range(NCORES)))

    out = np.empty((B, H, W, C), np.float32)
    for core in range(NCORES):
        b, half = divmod(core, 2)
        o = np.asarray(res.results[core]["out"]).astype(np.float32)
        o = o.reshape(C, HS, W)
        out[b, HS * half : HS * half + HS] = o.transpose(1, 2, 0)
    return out
